# revision 8
# baseline (speedup 1.0000x reference)
"""Distributed Trainium2 kernel for a GPT-style transformer block.

Strategy (8 NeuronCores):
 - Token-sharded (causal-load-balanced) LN1/QKV/attnproj/LN2/MLP: core c owns
   token chunks (c, 15-c) of each batch (512 tokens/core).
 - Head-sharded attention middle: AllToAll #1 exchanges qkv so core c holds
   heads (2c, 2c+1) for ALL tokens; uniform causal flash attention per head;
   AllToAll #2 returns attention output to token sharding.
 - Entropy: softmax rows export Z and Z2=sum(exp((1+eps)u)); host computes
   H = logZ - (logZ2 - logZ)/eps and averages.
 - bf16 matmuls (f32 accumulate), weights pre-transposed/pre-cast on host.
"""

import sys

sys.path.insert(0, "/opt/trn_rl_repo")

import numpy as np
import ml_dtypes

import concourse.bass as bass
import concourse.mybir as mybir
import concourse.tile as tile
from concourse import bacc
from concourse.bass_utils import run_bass_kernel_spmd
from concourse.masks import make_identity

bf16 = ml_dtypes.bfloat16
dt_bf = mybir.dt.bfloat16
dt_f32 = mybir.dt.float32
ALU = mybir.AluOpType
AF = mybir.ActivationFunctionType
AX = mybir.AxisListType

B, T, C, H, HD = 2, 2048, 1024, 16, 64
NCORES = 8
NCH = 16          # token chunks of 128 per batch
CHK = 128
TPC = 512         # tokens per core
EPS_E = 0.015625  # entropy derivative step
SC = 0.125        # 1/sqrt(HD)
LN_EPS = 1e-5

BLK1 = 196608     # a2a1 block elems: q(128x512) + k(128x512) + v(512x128)
BLK2 = 65536      # a2a2 block elems: 128x512


def _owner(m):
    return min(m, 15 - m)


def _slot(m):
    return 0 if m < 8 else 1


def _coloff(b, m):
    return 256 * b + 128 * _slot(m)


def _emit_ln(nc, pools, x_src, params_w, params_b, ident, eps_sb, out_tag):
    """LayerNorm of x_src [128, 4, 1024] f32 (tokens on partitions) ->
    8 transposed bf16 tiles [128 c, 512 t] with w,b applied. Returns list."""
    sbuf, psum, stats = pools
    xn_tiles = []
    for j in range(4):
        s1 = stats.tile([128, 1], dt_f32, tag="s1")
        nc.vector.reduce_sum(s1[:], x_src[:, j, :], axis=AX.X)
        mean = stats.tile([128, 1], dt_f32, tag="mean")
        nc.vector.tensor_scalar_mul(mean[:], s1[:], 1.0 / C)
        sq = sbuf.tile([128, 1024], dt_bf, tag="lnsq")
        s2 = stats.tile([128, 1], dt_f32, tag="s2")
        nc.scalar.activation(sq[:], x_src[:, j, :], AF.Square, accum_out=s2[:])
        ex2 = stats.tile([128, 1], dt_f32, tag="ex2")
        nc.vector.tensor_scalar_mul(ex2[:], s2[:], 1.0 / C)
        msq = stats.tile([128, 1], dt_f32, tag="msq")
        nc.vector.tensor_tensor(msq[:], mean[:], mean[:], ALU.mult)
        var = stats.tile([128, 1], dt_f32, tag="var")
        nc.vector.tensor_tensor(var[:], ex2[:], msq[:], ALU.subtract)
        std = stats.tile([128, 1], dt_f32, tag="std")
        nc.scalar.activation(std[:], var[:], AF.Sqrt, bias=eps_sb[:])
        rstd = stats.tile([128, 1], dt_f32, tag="rstd")
        nc.vector.reciprocal(rstd[:], std[:])
        xn = sbuf.tile([128, 1024], dt_bf, tag=f"xn{j}")
        nc.vector.tensor_scalar(
            xn[:], x_src[:, j, :], mean[:], rstd[:],
            op0=ALU.subtract, op1=ALU.mult,
        )
        xn_tiles.append(xn)
    outs = []
    for ci in range(8):
        ps_tr = psum.tile([128, 4, 128], dt_bf, tag="lntr")
        for j in range(4):
            nc.tensor.transpose(
                ps_tr[:, j, :], xn_tiles[j][:, 128 * ci : 128 * (ci + 1)], ident[:]
            )
        o = sbuf.tile([128, 512], dt_bf, tag=f"{out_tag}{ci}")
        nc.vector.tensor_scalar(
            o[:], ps_tr.rearrange("p a n -> p (a n)"),
            params_w[:, ci : ci + 1], params_b[:, ci : ci + 1],
            op0=ALU.mult, op1=ALU.add,
        )
        outs.append(o)
    return outs


def _build():
    nc = bacc.Bacc("TRN2", target_bir_lowering=False, debug=False, num_devices=NCORES)

    x_d = nc.dram_tensor("x", [TPC, C], dt_f32, kind="ExternalInput").ap()
    awt_d = nc.dram_tensor("attn_wt", [C, 3 * C], dt_bf, kind="ExternalInput").ap()
    ab_d = nc.dram_tensor("attn_b", [3 * C], dt_f32, kind="ExternalInput").ap()
    apw_d = nc.dram_tensor("attnproj_wt", [C, C], dt_bf, kind="ExternalInput").ap()
    apb_d = nc.dram_tensor("attnproj_b", [C], dt_f32, kind="ExternalInput").ap()
    fcw_d = nc.dram_tensor("fc_wt", [C, 4 * C], dt_bf, kind="ExternalInput").ap()
    fcb_d = nc.dram_tensor("fc_b", [4 * C], dt_f32, kind="ExternalInput").ap()
    pjw_d = nc.dram_tensor("proj_wt", [4 * C, C], dt_bf, kind="ExternalInput").ap()
    pjb_d = nc.dram_tensor("proj_b", [C], dt_f32, kind="ExternalInput").ap()
    l1w_d = nc.dram_tensor("ln1_w", [C], dt_f32, kind="ExternalInput").ap()
    l1b_d = nc.dram_tensor("ln1_b", [C], dt_f32, kind="ExternalInput").ap()
    l2w_d = nc.dram_tensor("ln2_w", [C], dt_f32, kind="ExternalInput").ap()
    l2b_d = nc.dram_tensor("ln2_b", [C], dt_f32, kind="ExternalInput").ap()
    cm_d = nc.dram_tensor("cmask", [128, 128], dt_f32, kind="ExternalInput").ap()

    out_d = nc.dram_tensor("out", [TPC, C], dt_f32, kind="ExternalOutput").ap()
    zs_d = nc.dram_tensor("zs", [2, 2, 16, 2, 128], dt_f32, kind="ExternalOutput").ap()

    rg = [list(range(NCORES))]

    with tile.TileContext(nc) as tc:
        with (
            tc.tile_pool(name="consts", bufs=1) as consts,
            tc.tile_pool(name="persist", bufs=1) as persist,
            tc.tile_pool(name="dram", bufs=1, space="DRAM") as dram,
        ):
            ident = consts.tile([128, 128], dt_bf)
            make_identity(nc, ident[:])
            eps_sb = consts.tile([128, 1], dt_f32)
            nc.vector.memset(eps_sb[:], LN_EPS)
            cmask = consts.tile([128, 128], dt_f32)
            nc.sync.dma_start(cmask[:], cm_d[:])
            ab_sb = consts.tile([128, 24], dt_f32)
            nc.sync.dma_start(ab_sb[:], ab_d.rearrange("(a p) -> p a", p=128))
            apb_sb = consts.tile([128, 8], dt_f32)
            nc.sync.dma_start(apb_sb[:], apb_d.rearrange("(a p) -> p a", p=128))
            fcb_sb = consts.tile([128, 32], dt_f32)
            nc.sync.dma_start(fcb_sb[:], fcb_d.rearrange("(a p) -> p a", p=128))
            pjb_sb = consts.tile([128, 8], dt_f32)
            nc.sync.dma_start(pjb_sb[:], pjb_d.rearrange("(a p) -> p a", p=128))
            l1w_sb = consts.tile([128, 8], dt_f32)
            nc.sync.dma_start(l1w_sb[:], l1w_d.rearrange("(a p) -> p a", p=128))
            l1b_sb = consts.tile([128, 8], dt_f32)
            nc.sync.dma_start(l1b_sb[:], l1b_d.rearrange("(a p) -> p a", p=128))
            l2w_sb = consts.tile([128, 8], dt_f32)
            nc.sync.dma_start(l2w_sb[:], l2w_d.rearrange("(a p) -> p a", p=128))
            l2b_sb = consts.tile([128, 8], dt_f32)
            nc.sync.dma_start(l2b_sb[:], l2b_d.rearrange("(a p) -> p a", p=128))

            x_sb = persist.tile([128, 4, 1024], dt_f32)
            nc.sync.dma_start(x_sb[:], x_d.rearrange("(a p) c -> p a c", p=128))
            x1_sb = persist.tile([128, 4, 1024], dt_f32)

            # ---------- LN1 + QKV + A2A1 build ----------
            a2a1_in = dram.tile([NCORES, BLK1], dt_bf)
            a2a1_out = dram.tile([NCORES, BLK1], dt_bf)
            with (
                tc.tile_pool(name="p1sb", bufs=1) as p1sb,
                tc.tile_pool(name="p1ps", bufs=2, space="PSUM") as p1ps,
                tc.tile_pool(name="p1st", bufs=2) as p1st,
            ):
                ln1xT = _emit_ln(
                    nc, (p1sb, p1ps, p1st), x_sb, l1w_sb, l1b_sb, ident, eps_sb, "l1x"
                )
                awt_sb = p1sb.tile([128, 8, 3 * C], dt_bf)
                nc.sync.dma_start(
                    awt_sb[:], awt_d.rearrange("(a p) n -> p a n", p=128)
                )
                qkvT = p1sb.tile([128, 24, 512], dt_bf)
                for oc in range(24):
                    ps = p1ps.tile([128, 512], dt_f32, tag="qkvps")
                    for ci in range(8):
                        nc.tensor.matmul(
                            ps[:],
                            lhsT=awt_sb[:, ci, 128 * oc : 128 * (oc + 1)],
                            rhs=ln1xT[ci][:],
                            start=(ci == 0),
                            stop=(ci == 7),
                        )
                    nc.scalar.activation(
                        qkvT[:, oc, :], ps[:], AF.Identity,
                        bias=ab_sb[:, oc : oc + 1],
                    )
                # a2a1 payload: per dest d: q chunk d, k chunk d, v chunk d transposed
                for d in range(8):
                    nc.sync.dma_start(
                        a2a1_in[d, 0:65536].rearrange("(p n) -> p n", p=128),
                        qkvT[:, d, :],
                    )
                    nc.sync.dma_start(
                        a2a1_in[d, 65536:131072].rearrange("(p n) -> p n", p=128),
                        qkvT[:, 8 + d, :],
                    )
                    for blk in range(4):
                        ps_v = p1ps.tile([128, 128], dt_bf, tag="vtr")
                        nc.tensor.transpose(
                            ps_v[:],
                            qkvT[:, 16 + d, 128 * blk : 128 * (blk + 1)],
                            ident[:],
                        )
                        vnat = p1sb.tile([128, 128], dt_bf, tag="vnat", bufs=3)
                        nc.vector.tensor_copy(vnat[:], ps_v[:])
                        off = 131072 + 16384 * blk
                        nc.sync.dma_start(
                            a2a1_in[d, off : off + 16384].rearrange(
                                "(p n) -> p n", p=128
                            ),
                            vnat[:],
                        )
            nc.gpsimd.collective_compute(
                "AllToAll", ALU.bypass, replica_groups=rg,
                ins=[a2a1_in.opt()], outs=[a2a1_out.opt()],
            )

            # ---------- attention (heads 2c, 2c+1 over all tokens) ----------
            a2a2_in = dram.tile([NCORES, BLK2], dt_bf)
            a2a2_out = dram.tile([NCORES, BLK2], dt_bf)
            with (
                tc.tile_pool(name="akv", bufs=2) as akv,
                tc.tile_pool(name="asb", bufs=3) as asb,
                tc.tile_pool(name="azs", bufs=4) as azs,
                tc.tile_pool(name="aps_s", bufs=2, space="PSUM") as aps_s,
                tc.tile_pool(name="aps_y", bufs=2, space="PSUM") as aps_y,
                tc.tile_pool(name="aps_z", bufs=2, space="PSUM") as aps_z,
            ):
                for b in range(2):
                    for hl in range(2):
                        kt = akv.tile([64, 2048], dt_bf, tag="kt")
                        for m in range(16):
                            j = _owner(m)
                            base = 65536 + (64 * hl) * 512
                            src = a2a1_out[j, base : base + 64 * 512].rearrange(
                                "(p n) -> p n", p=64
                            )[:, _coloff(b, m) : _coloff(b, m) + 128]
                            nc.sync.dma_start(kt[:, 128 * m : 128 * (m + 1)], src)
                        vx = akv.tile([128, 16, 65], dt_bf, tag="vx")
                        nc.vector.memset(vx[:], 1.0)
                        for m in range(16):
                            j = _owner(m)
                            src = a2a1_out[
                                j, 131072 : 131072 + 65536
                            ].rearrange("(p n) -> p n", p=512)[
                                _coloff(b, m) : _coloff(b, m) + 128,
                                64 * hl : 64 * hl + 64,
                            ]
                            nc.sync.dma_start(vx[:, m, 0:64], src)
                        for r in range(16):
                            j = _owner(r)
                            qt = asb.tile([64, 128], dt_bf, tag="qt")
                            src = a2a1_out[j, (64 * hl) * 512 : (64 * hl + 64) * 512]
                            src = src.rearrange("(p n) -> p n", p=64)[
                                :, _coloff(b, r) : _coloff(b, r) + 128
                            ]
                            nc.sync.dma_start(qt[:], src)
                            nblk = r + 1
                            ps_y = aps_y.tile([65, 128], dt_f32, tag="psy")
                            ps_z2 = aps_z.tile([65, 128], dt_f32, tag="psz")
                            for g0 in range(0, nblk, 8):
                                gn = min(8, nblk - g0)
                                ps_s = aps_s.tile([128, 8, 128], dt_f32, tag="pss")
                                for mi in range(gn):
                                    m = g0 + mi
                                    nc.tensor.matmul(
                                        ps_s[:, mi, :],
                                        lhsT=kt[:, 128 * m : 128 * (m + 1)],
                                        rhs=qt[:],
                                        start=True, stop=True,
                                    )
                                if g0 <= r < g0 + gn:
                                    mi_d = r - g0
                                    nc.vector.tensor_tensor(
                                        ps_s[:, mi_d, :], ps_s[:, mi_d, :],
                                        cmask[:], ALU.add,
                                    )
                                gw = gn * 128
                                flat = ps_s.rearrange("p a n -> p (a n)")
                                ptg = asb.tile([128, 1024], dt_bf, tag="ptg")
                                p2g = asb.tile([128, 1024], dt_bf, tag="p2g")
                                nc.scalar.activation(
                                    ptg[:, :gw], flat[:, :gw], AF.Exp, scale=SC
                                )
                                nc.scalar.activation(
                                    p2g[:, :gw], flat[:, :gw], AF.Exp,
                                    scale=SC * (1.0 + EPS_E),
                                )
                                for mi in range(gn):
                                    m = g0 + mi
                                    nc.tensor.matmul(
                                        ps_y[:], lhsT=vx[:, m, :],
                                        rhs=ptg[:, 128 * mi : 128 * (mi + 1)],
                                        start=(m == 0), stop=(m == r),
                                    )
                                    nc.tensor.matmul(
                                        ps_z2[:], lhsT=vx[:, m, :],
                                        rhs=p2g[:, 128 * mi : 128 * (mi + 1)],
                                        start=(m == 0), stop=(m == r),
                                    )
                            zA = azs.tile([1, 128], dt_f32, tag="zA")
                            nc.vector.tensor_copy(zA[:], ps_y[64:65, :])
                            nc.sync.dma_start(zs_d[b, hl, r, 0:1, :], zA[:])
                            zB = azs.tile([1, 128], dt_f32, tag="zB")
                            nc.vector.tensor_copy(zB[:], ps_z2[64:65, :])
                            nc.sync.dma_start(zs_d[b, hl, r, 1:2, :], zB[:])
                            rz = azs.tile([1, 128], dt_f32, tag="rz")
                            nc.vector.reciprocal(rz[:], ps_y[64:65, :])
                            zb = azs.tile([64, 128], dt_f32, tag="zb")
                            nc.gpsimd.partition_broadcast(zb[:], rz[:])
                            yn = asb.tile([64, 128], dt_bf, tag="yn")
                            with nc.allow_low_precision(reason="attn out bf16"):
                                nc.vector.tensor_tensor(
                                    yn[:], ps_y[0:64, :], zb[:], ALU.mult
                                )
                            dst = a2a2_in[
                                _owner(r),
                                (64 * hl) * 512 : (64 * hl + 64) * 512,
                            ].rearrange("(p n) -> p n", p=64)[
                                :, _coloff(b, r) : _coloff(b, r) + 128
                            ]
                            nc.sync.dma_start(dst, yn[:])
            nc.gpsimd.collective_compute(
                "AllToAll", ALU.bypass, replica_groups=rg,
                ins=[a2a2_in.opt()], outs=[a2a2_out.opt()],
            )

            # ---------- attnproj + residual ----------
            with (
                tc.tile_pool(name="p5sb", bufs=1) as p5sb,
                tc.tile_pool(name="p5ps", bufs=2, space="PSUM") as p5ps,
            ):
                yat = p5sb.tile([128, 8, 512], dt_bf)
                for j in range(8):
                    nc.sync.dma_start(
                        yat[:, j, :],
                        a2a2_out[j, :].rearrange("(p n) -> p n", p=128),
                    )
                apw_sb = p5sb.tile([128, 8, C], dt_bf)
                nc.sync.dma_start(
                    apw_sb[:], apw_d.rearrange("(a p) n -> p a n", p=128)
                )
                y1T = []
                for oc in range(8):
                    ps = p5ps.tile([128, 512], dt_f32, tag="appps")
                    for ci in range(8):
                        nc.tensor.matmul(
                            ps[:],
                            lhsT=apw_sb[:, ci, 128 * oc : 128 * (oc + 1)],
                            rhs=yat[:, ci, :],
                            start=(ci == 0), stop=(ci == 7),
                        )
                    o = p5sb.tile([128, 512], dt_bf, tag=f"y1T{oc}")
                    nc.scalar.activation(
                        o[:], ps[:], AF.Identity, bias=apb_sb[:, oc : oc + 1]
                    )
                    y1T.append(o)
                for j in range(4):
                    ps1 = p5ps.tile([128, 8, 128], dt_bf, tag="y1tr")
                    for oc in range(8):
                        nc.tensor.transpose(
                            ps1[:, oc, :], y1T[oc][:, 128 * j : 128 * (j + 1)],
                            ident[:],
                        )
                    nc.vector.tensor_tensor(
                        x1_sb[:, j, :], x_sb[:, j, :],
                        ps1.rearrange("p a n -> p (a n)"), ALU.add,
                    )

            # ---------- LN2 + MLP fc/gelu ----------
            with tc.tile_pool(name="pmg", bufs=1) as pmg:
              with (
                tc.tile_pool(name="p7sb", bufs=1) as p7sb,
                tc.tile_pool(name="p7ps", bufs=2, space="PSUM") as p7ps,
                tc.tile_pool(name="p7st", bufs=2) as p7st,
              ):
                ln2xT = _emit_ln(
                    nc, (p7sb, p7ps, p7st), x1_sb, l2w_sb, l2b_sb, ident, eps_sb, "l2x"
                )
                fcw_sb = p7sb.tile([128, 8, 4 * C], dt_bf)
                nc.sync.dma_start(
                    fcw_sb[:], fcw_d.rearrange("(a p) n -> p a n", p=128)
                )
                mg = []
                for oc in range(32):
                    ps = p7ps.tile([128, 512], dt_f32, tag="fcps")
                    for ci in range(8):
                        nc.tensor.matmul(
                            ps[:],
                            lhsT=fcw_sb[:, ci, 128 * oc : 128 * (oc + 1)],
                            rhs=ln2xT[ci][:],
                            start=(ci == 0), stop=(ci == 7),
                        )
                    o = pmg.tile([128, 512], dt_bf, tag=f"mg{oc}")
                    nc.scalar.activation(
                        o[:], ps[:], AF.Gelu_apprx_tanh,
                        bias=fcb_sb[:, oc : oc + 1],
                    )
                    mg.append(o)

              # ---------- proj + final residual ----------
              with (
                    tc.tile_pool(name="p8sb", bufs=1) as p8sb,
                    tc.tile_pool(name="p8ps", bufs=2, space="PSUM") as p8ps,
                ):
                    pjw_sb = p8sb.tile([128, 32, C], dt_bf)
                    nc.sync.dma_start(
                        pjw_sb[:], pjw_d.rearrange("(a p) n -> p a n", p=128)
                    )
                    y3T = []
                    for oc in range(8):
                        ps = p8ps.tile([128, 512], dt_f32, tag="pjps")
                        for ci in range(32):
                            nc.tensor.matmul(
                                ps[:],
                                lhsT=pjw_sb[:, ci, 128 * oc : 128 * (oc + 1)],
                                rhs=mg[ci][:],
                                start=(ci == 0), stop=(ci == 31),
                            )
                        o = p8sb.tile([128, 512], dt_bf, tag=f"y3T{oc}")
                        nc.scalar.activation(
                            o[:], ps[:], AF.Identity, bias=pjb_sb[:, oc : oc + 1]
                        )
                        y3T.append(o)
                    out_sb = p8sb.tile([128, 4, 1024], dt_f32)
                    for j in range(4):
                        ps3 = p8ps.tile([128, 8, 128], dt_bf, tag="y3tr")
                        for oc in range(8):
                            nc.tensor.transpose(
                                ps3[:, oc, :],
                                y3T[oc][:, 128 * j : 128 * (j + 1)], ident[:],
                            )
                        nc.vector.tensor_tensor(
                            out_sb[:, j, :], x1_sb[:, j, :],
                            ps3.rearrange("p a n -> p (a n)"), ALU.add,
                        )
                    nc.sync.dma_start(
                        out_d.rearrange("(a p) c -> p a c", p=128), out_sb[:]
                    )

    nc.compile()
    return nc


_CACHE = {}
_LAST_RES = None


def _get_nc():
    if "nc" not in _CACHE:
        _CACHE["nc"] = _build()
    return _CACHE["nc"]


def kernel(x, ln1_w, ln1_b, attn_w, attn_b, attnproj_w, attnproj_b,
           ln2_w, ln2_b, fc_w, fc_b, proj_w, proj_b):
    x = np.asarray(x, np.float32)
    nc = _get_nc()

    awt = np.ascontiguousarray(np.asarray(attn_w, np.float32).T).astype(bf16)
    apwt = np.ascontiguousarray(np.asarray(attnproj_w, np.float32).T).astype(bf16)
    fcwt = np.ascontiguousarray(np.asarray(fc_w, np.float32).T).astype(bf16)
    pjwt = np.ascontiguousarray(np.asarray(proj_w, np.float32).T).astype(bf16)
    cm = np.where(np.triu(np.ones((128, 128), bool)), 0.0, -30000.0).astype(np.float32)

    shared = {
        "attn_wt": awt, "attn_b": np.asarray(attn_b, np.float32),
        "attnproj_wt": apwt, "attnproj_b": np.asarray(attnproj_b, np.float32),
        "fc_wt": fcwt, "fc_b": np.asarray(fc_b, np.float32),
        "proj_wt": pjwt, "proj_b": np.asarray(proj_b, np.float32),
        "ln1_w": np.asarray(ln1_w, np.float32), "ln1_b": np.asarray(ln1_b, np.float32),
        "ln2_w": np.asarray(ln2_w, np.float32), "ln2_b": np.asarray(ln2_b, np.float32),
        "cmask": cm,
    }
    in_maps = []
    for c in range(NCORES):
        cA, cB = c, 15 - c
        xs = np.concatenate(
            [
                x[0, 128 * cA : 128 * (cA + 1)],
                x[0, 128 * cB : 128 * (cB + 1)],
                x[1, 128 * cA : 128 * (cA + 1)],
                x[1, 128 * cB : 128 * (cB + 1)],
            ],
            axis=0,
        )
        in_maps.append({**shared, "x": np.ascontiguousarray(xs)})

    res = run_bass_kernel_spmd(nc, in_maps, list(range(NCORES)))
    global _LAST_RES
    _LAST_RES = res

    x_out = np.empty((B, T, C), np.float32)
    hsum = 0.0
    for c in range(NCORES):
        o = res.results[c]["out"]
        cA, cB = c, 15 - c
        x_out[0, 128 * cA : 128 * (cA + 1)] = o[0:128]
        x_out[0, 128 * cB : 128 * (cB + 1)] = o[128:256]
        x_out[1, 128 * cA : 128 * (cA + 1)] = o[256:384]
        x_out[1, 128 * cB : 128 * (cB + 1)] = o[384:512]
        zs = res.results[c]["zs"].astype(np.float64)
        Z = zs[:, :, :, 0, :]
        Z2 = zs[:, :, :, 1, :]
        lZ = np.log(Z)
        hrow = lZ - (np.log(Z2) - lZ) / EPS_E
        hsum += hrow.sum()
    entropy = np.float32(hsum / (B * H * T))
    return x_out, entropy
